# revision 2
# baseline (speedup 1.0000x reference)
"""DINO loss kernel for Trainium2 (8 NeuronCores, Bass/Tile) — v2.

Math (identical factorization to v1)
------------------------------------
With q = log_softmax(student/ts) [Ns=1280, D] and
p = softmax((teacher-center)/tt) [Nt=256, D]:

    loss = sum_{i != j} ( -sum_d p[i,d] q[j,d] ) / (Nt*Ns - Nt)

    sum_{i,j} ce[i,j] = -(dot(P,S)/ts - C*sum(P))
      P[d] = sum_i p[i,d]                (teacher prob column sums)
      S[d] = sum_j x_s[j,d]              (raw student logit column sums)
      C    = sum_j logsumexp(x_s[j]/ts)
    diag  = sum_i v_i/(ts*Z_i) - C_g     (v_i = sum_d e_t[i,d]*sg[i,d])
    loss  = ( -(P.S/ts - C*sum(P)) + diag ) / (Nt*Ns - Nt)

v2 changes vs v1 (216us -> target ~95us cost-model)
---------------------------------------------------
* All inputs stream to the device as bf16: halves DMA bytes (48->24 MB
  per core). Error budget: student logsumexp only sees input rounding
  (ACT accumulates exp sums in f32 pre-rounding); measured end-to-end
  rel err ~1e-4 vs the 2e-2 gate.
* Teacher exp bias is a host-sampled bound (like the student rows in
  v1) instead of an exact on-device max: removes the cross-partition
  max pass and its serializing dependency. A common bound cancels in
  v/Z and in P (e scaled uniformly, Z likewise); overflow cannot happen
  below a ~3.5 sampling gap and non-finite stats trigger the exact
  numpy fallback anyway.
* Column sums via bf16 mask-matmuls whose one-hot lhsT is a sliding
  window into a tiny constant buffer; each 512-col slice lands on its
  own PSUM partition (32-row quadrant groups at bases 0/32/64/96), so
  the whole per-core colsum output is ONE [128,512] PSUM bank retired
  by ONE DVE copy + one DMA (v1: 48 copies + 48 DMAs).
* Student exp outputs are throwaway (accum_out only): sl exps run
  in-place over the arrival buffer, sg exps into one scratch tile.
* Two pipelined DMA queues (SP hwdge for the sl stream, Pool swdge for
  teacher/sg/stats): per-DMA fixed costs overlap with transfers.
"""

import numpy as np
import ml_dtypes

import concourse.bass as bass
import concourse.bacc as bacc
import concourse.tile as tile
from concourse import mybir
from concourse.bass_utils import run_bass_kernel_spmd

BF16 = mybir.dt.bfloat16
F32 = mybir.dt.float32
AX = mybir.AxisListType
EXP = mybir.ActivationFunctionType.Exp

N_CORES = 8
D = 65536
N_T = 256
N_G = 256
N_L = 1024
SL_ROWS = N_L // N_CORES          # 128 student_local rows per core
SG_ROWS = N_G // N_CORES          # 32 student_global rows per core
T_ROWS = N_T // N_CORES           # 32 teacher rows per core
DQ = D // 4                       # quarter-cols for row-split x4 tensors

REG = 512                         # matmul moving free size
SL_CHUNK = 8192                   # sl cols per chunk (8 chunks)
N_SL_CH = D // SL_CHUNK
QT_CHUNK = 4096                   # t/sg quarter-cols per chunk (4 chunks)
N_QT_CH = DQ // QT_CHUNK


def _masks():
    """Sliding-window mask buffers (bf16).

    b1[p, c] = 1 iff c == 31.  Window b1[:, 31-m : 63-m] is a [128, 32]
    lhsT whose only nonzero column is m -> colsum over all 128
    partitions lands on out row m.

    b2[p, c] = 1 iff c == 28 + p%4.  Window b2[:, 28-4s : 60-4s] is a
    lhsT with ones at (p, 4s + p%4) -> per-quarter colsums of one slice
    land on out rows 4s..4s+3.

    fm[p, i] = 1 iff i == p//4: matmul lhsT folding a [128,1] column
    into [32,1] row-groups (cross-partition reduce without a DMA).
    bm[q, p] = 1 iff p//4 == q: matmul lhsT broadcasting a [32,1]
    column back to [128,1].
    """
    b1 = np.zeros((128, 64), ml_dtypes.bfloat16)
    b1[:, 31] = 1.0
    b2 = np.zeros((128, 64), ml_dtypes.bfloat16)
    b2[np.arange(128), 28 + np.arange(128) % 4] = 1.0
    fm = np.zeros((128, 32), ml_dtypes.bfloat16)
    fm[np.arange(128), np.arange(128) // 4] = 1.0
    bm = np.zeros((32, 128), ml_dtypes.bfloat16)
    bm[np.arange(128) // 4, np.arange(128)] = 1.0
    return b1, b2, fm, bm


def build_nc(ts=0.1, tt=0.04, variant="full"):
    """Per-core Bass program; all 8 cores run this same NEFF.

    variant: timing-experiment knob ("full" for the real kernel):
      dma_only  — input DMAs only
      no_act    — everything except the ACT exps (+stats dma dropped)
      act_only  — DMAs + ACT exps only
    """
    v_dma = variant == "dma_only"
    v_noact = variant == "no_act"
    v_actonly = variant == "act_only"
    nc = bacc.Bacc()
    sl = nc.dram_tensor("sl", [128, D], BF16, kind="ExternalInput")
    sg = nc.dram_tensor("sg", [128, DQ], BF16, kind="ExternalInput")
    t = nc.dram_tensor("t", [128, DQ], BF16, kind="ExternalInput")
    nbs = nc.dram_tensor("nbs", [128, 1], F32, kind="ExternalInput")
    nbt = nc.dram_tensor("nbt", [128, 1], F32, kind="ExternalInput")

    b1_np, b2_np, fm_np, bm_np = _masks()
    b1_d = nc.inline_tensor(b1_np, name="b1_c")
    b2_d = nc.inline_tensor(b2_np, name="b2_c")
    fm_d = nc.inline_tensor(fm_np, name="fm_c")
    bm_d = nc.inline_tensor(bm_np, name="bm_c")

    s_sl = nc.dram_tensor("s_sl", [128, REG], F32, kind="ExternalOutput")
    s_sg = nc.dram_tensor("s_sg", [128, REG], F32, kind="ExternalOutput")
    p_out = nc.dram_tensor("p_out", [128, REG], F32, kind="ExternalOutput")
    stats = nc.dram_tensor("stats", [128, 19], F32, kind="ExternalOutput")
    # stats cols: 0:9 w_sl, 9:11 w_sg, 11:15 z_t, 15:19 v_t

    with tile.TileContext(nc) as tc:
        with (
            tc.tile_pool(name="singles", bufs=1) as singles,
            tc.tile_pool(name="big", bufs=1) as big,
            tc.tile_pool(name="slch", bufs=5) as slch,
            tc.tile_pool(name="stats", bufs=1) as stp,
            tc.tile_pool(name="ps", bufs=1, space="PSUM") as psp,
        ):
            # Warm the ACT exp table at t~0 so the 1.3us LoadActFuncSet
            # doesn't sit in front of the first real exp.
            warm = singles.tile([128, 1], F32)
            nc.vector.memset(warm, 0)
            nc.scalar.activation(warm, warm, EXP)

            # --- small constants, all on the Pool (swdge) queue so the
            # SP queue's first transfer is the first teacher chunk.
            # Order: biases first (ACT needs nbt at ~4.5us), masks next
            # (PE needs b1 at ~7us), fold masks last (needed at ~40us).
            nbt_t = singles.tile([128, 1], F32)
            nc.gpsimd.dma_start(out=nbt_t, in_=nbt[:, :])
            nbs_t = singles.tile([128, 1], F32)
            nc.gpsimd.dma_start(out=nbs_t, in_=nbs[:, :])
            b1t = singles.tile([128, 64], BF16)
            nc.gpsimd.dma_start(out=b1t, in_=b1_d[:, :])
            b2t = singles.tile([128, 64], BF16)
            nc.gpsimd.dma_start(out=b2t, in_=b2_d[:, :])
            fmt = singles.tile([128, 32], BF16)
            nc.gpsimd.dma_start(out=fmt, in_=fm_d[:, :])
            bmt = singles.tile([32, 128], BF16)
            nc.gpsimd.dma_start(out=bmt, in_=bm_d[:, :])

            # --- resident tensors and scratch
            tt_tile = big.tile([128, DQ], BF16)       # teacher (exp'd in place)
            sg_tile = big.tile([128, DQ], BF16)       # sg raw
            esc = big.tile([128, 8192], BF16)         # sg exp scratch
            vsc = big.tile([128, QT_CHUNK], BF16)     # vhat product scratch
            # sl exp scratch: NOT in-place over the arrival buffer — that
            # would chain ACT behind the chunk's PE colsum matmuls (WAR),
            # and PE (p-state-limited) is slower per chunk than ACT.
            lsc = big.tile([128, SL_CHUNK], BF16)
            stats_t = stp.tile([128, 19], F32)
            w_sl = stats_t[:, 0:9]
            w_sg = stats_t[:, 9:11]
            z_t = stats_t[:, 11:15]
            v_t = stats_t[:, 15:19]

            ps_sl = psp.tile([128, REG], F32)
            ps_sg = psp.tile([128, REG], F32)
            ps_p = psp.tile([128, REG], F32)

            # --- input DMAs: ALL on the SP queue, emitted in exactly the
            # order ACT consumes them, so arrival order == need order.
            def t_dma(j):
                nc.sync.dma_start(
                    out=tt_tile[:, j * QT_CHUNK:(j + 1) * QT_CHUNK],
                    in_=t[:, j * QT_CHUNK:(j + 1) * QT_CHUNK])

            def sg_dma(j):
                nc.sync.dma_start(
                    out=sg_tile[:, j * QT_CHUNK:(j + 1) * QT_CHUNK],
                    in_=sg[:, j * QT_CHUNK:(j + 1) * QT_CHUNK])

            # --- ACT pieces
            def t_exp(j):
                if v_dma or v_noact:
                    return
                sel = tt_tile[:, j * QT_CHUNK:(j + 1) * QT_CHUNK]
                nc.scalar.activation(sel, sel, EXP, bias=nbt_t, scale=1.0 / tt,
                                     accum_out=z_t[:, j:j + 1])

            def sg_exp(h):
                # h in {0, 1}: one [128, 8192] exp per sg half.
                if v_dma or v_noact:
                    return
                nc.scalar.activation(
                    esc, sg_tile[:, h * 8192:(h + 1) * 8192],
                    EXP, bias=nbs_t, scale=1.0 / ts,
                    accum_out=w_sg[:, h:h + 1])

            # --- PE pieces
            def sg_mm(j):
                if v_dma or v_actonly:
                    return
                # 8 slices per chunk; slice index within sg: 8j..8j+7.
                for s8 in range(QT_CHUNK // REG):
                    sli = j * (QT_CHUNK // REG) + s8      # 0..31
                    r, s = sli // 8, sli % 8
                    nc.tensor.matmul(
                        ps_sg[32 * r:32 * r + 32, :],
                        b2t[:, 28 - 4 * s:60 - 4 * s],
                        sg_tile[:, sli * REG:(sli + 1) * REG],
                        start=(s == 0), stop=(s == 7),
                        tile_position=(0, 32 * r),
                        skip_group_check=True)

            def p_mm(wt, j):
                if variant != "full":
                    return
                for s8 in range(QT_CHUNK // REG):
                    sli = j * (QT_CHUNK // REG) + s8
                    r, s = sli // 8, sli % 8
                    nc.tensor.matmul(
                        ps_p[32 * r:32 * r + 32, :],
                        wt[:, 28 - 4 * s:60 - 4 * s],
                        tt_tile[:, sli * REG:(sli + 1) * REG],
                        start=(s == 0), stop=(s == 7),
                        tile_position=(0, 32 * r),
                        skip_group_check=True)

            # --- the sl stream (SP queue, chunk pool, in-place exp).
            # Chunk = slices [sli0, sli0+nsli); masks depend only on the
            # global slice index so chunk sizes are free to vary.
            def sl_chunk(sli0, nsli, k):
                ch = slch.tile([128, SL_CHUNK], BF16, tag="slch")
                chv = ch[:, 0:nsli * REG]
                # halve big-chunk DMAs so PE can start on the first half
                # ~3us earlier (PE waits the full transfer sem otherwise)
                if nsli > 8:
                    h = (nsli // 2) * REG
                    nc.sync.dma_start(
                        out=ch[:, 0:h], in_=sl[:, sli0 * REG:sli0 * REG + h])
                    nc.sync.dma_start(
                        out=ch[:, h:nsli * REG],
                        in_=sl[:, sli0 * REG + h:(sli0 + nsli) * REG])
                else:
                    nc.sync.dma_start(
                        out=chv, in_=sl[:, sli0 * REG:(sli0 + nsli) * REG])
                if not (v_dma or v_actonly):
                    for s16 in range(nsli):
                        sli = sli0 + s16                      # 0..127
                        m = sli % 32
                        nc.tensor.matmul(
                            ps_sl[32 * (sli // 32):32 * (sli // 32) + 32, :],
                            b1t[:, 31 - m:63 - m],
                            ch[:, s16 * REG:(s16 + 1) * REG],
                            start=(m == 0), stop=(m == 31),
                            tile_position=(0, 32 * (sli // 32)),
                            skip_group_check=True)
                if not (v_dma or v_noact):
                    nc.scalar.activation(lsc[:, 0:nsli * REG], chv, EXP,
                                         bias=nbs_t, scale=1.0 / ts,
                                         accum_out=w_sl[:, k:k + 1])

            # --- DVE pieces
            def vhat(j):
                sel_e = tt_tile[:, j * QT_CHUNK:(j + 1) * QT_CHUNK]
                sel_g = sg_tile[:, j * QT_CHUNK:(j + 1) * QT_CHUNK]
                nc.vector.tensor_mul(vsc, sel_e, sel_g)
                nc.vector.reduce_sum(v_t[:, j:j + 1], vsc, axis=AX.X)

            # ---------------- emission schedule ----------------
            # One SP input queue in ACT consumption order; the first two
            # sl chunks are half-size so ACT starts early and the PE
            # p-state ramps on small bursts.  9 sl chunks: 8,8 then 7x16
            # slices.  DMA+PE+ACT for each chunk are emitted together;
            # per-engine order is the call order below.
            t_dma(0)
            t_exp(0)
            sl_chunk(0, 8, 0)
            t_dma(1)
            t_exp(1)
            sg_dma(0)
            sg_mm(0)
            sl_chunk(8, 8, 1)
            t_dma(2)
            t_exp(2)
            sg_dma(1)
            sg_exp(0)
            sg_mm(1)
            sl_chunk(16, 16, 2)
            t_dma(3)
            t_exp(3)
            sg_dma(2)
            sg_mm(2)
            sl_chunk(32, 16, 3)
            sg_dma(3)
            sg_exp(1)
            sg_mm(3)

            # Z fold entirely on-core (no DMAs — tiny fold DMAs queue
            # behind bulk transfers on the DMA device and arrive ~20us
            # late): PE matmuls do the cross-partition fold/broadcast.
            #   zloc[p]  = sum_j z_t[p, j]                (DVE, bf16 out)
            #   z32[i]   = sum_{p//4==i} zloc[p]          (PE: fmask)
            #   rz32     = 1/z32                          (DVE, bf16 out)
            #   rzb[p]   = rz32[p//4]                     (PE: bmask)
            #   wt       = b2 * rzb                       (DVE)
            wt = None
            if variant == "full":
                with nc.allow_low_precision(
                        reason="1/Z weights carry ~0.4% bf16 noise; the "
                               "loss-level effect is ~1e-4 (budget 2e-2)"):
                    zloc = stp.tile([128, 1], BF16)
                    nc.vector.reduce_sum(zloc, z_t, axis=AX.X)
                    z32p = psp.tile([32, 1], F32)
                    nc.tensor.matmul(z32p, fmt[:, 0:32], zloc,
                                     start=True, stop=True)
                    rz32 = stp.tile([32, 1], BF16)
                    nc.vector.reciprocal(rz32, z32p)
                    rzbp = psp.tile([128, 1], F32)
                    nc.tensor.matmul(rzbp, bmt[:, 0:128], rz32,
                                     start=True, stop=True)
                    wt = stp.tile([128, 64], BF16)
                    nc.vector.tensor_scalar_mul(wt, b2t, rzbp)

            if variant == "full":
                vhat(0)
                vhat(1)
                vhat(2)
                vhat(3)

            # P colsums can interleave with the sl tail: the scheduler is
            # greedy-by-readiness per engine, so if wt isn't ready yet PE
            # just runs later-priority sl matmuls instead of stalling.
            sl_chunk(48, 16, 4)
            p_mm(wt, 0)
            p_mm(wt, 1)
            sl_chunk(64, 16, 5)
            p_mm(wt, 2)
            p_mm(wt, 3)
            sl_chunk(80, 16, 6)
            sl_chunk(96, 16, 7)
            sl_chunk(112, 16, 8)

            # ---- retires: one DVE copy + one Pool DMA per PSUM bank
            def retire(ps_t, dst):
                st = stp.tile([128, REG], F32, tag=f"st_{dst.name}")
                nc.vector.tensor_copy(out=st, in_=ps_t)
                nc.gpsimd.dma_start(out=dst[:, :], in_=st)

            if variant == "full":
                retire(ps_sg, s_sg)
                retire(ps_p, p_out)
                retire(ps_sl, s_sl)
                nc.sync.dma_start(out=stats[:, :], in_=stats_t)

    nc.compile()
    return nc


_NC_CACHE = {}


def _get_nc(ts, tt):
    key = (round(ts, 9), round(tt, 9))
    if key not in _NC_CACHE:
        _NC_CACHE[key] = build_nc(ts=ts, tt=tt)
    return _NC_CACHE[key]


def _decode_quarters(a):
    """[128, 512] psum layout -> [4, 16384] (quarter, quarter-col)."""
    return (a.reshape(4, 8, 4, REG).transpose(2, 0, 1, 3).reshape(4, DQ))


def _merge(results, ts, tt, bs_scaled):
    """Host-side exact merge of per-core device outputs (float64)."""
    S = np.zeros(D, np.float64)
    P = np.zeros(D, np.float64)
    C = 0.0
    C_g = 0.0
    diag1 = 0.0
    healthy = True
    for r in results:
        s_sl = r["s_sl"].astype(np.float64)
        s_sg = _decode_quarters(r["s_sg"].astype(np.float64))
        p_o = _decode_quarters(r["p_out"].astype(np.float64))
        S += s_sl.reshape(-1)
        S += s_sg.reshape(-1)
        P += p_o.reshape(-1)
        st = r["stats"].astype(np.float64)
        w_sl, w_sg = st[:, 0:9], st[:, 9:11]
        z_t, v_t = st[:, 11:15], st[:, 15:19]
        wsum = w_sl.sum(axis=1)
        healthy &= bool(np.isfinite(w_sl).all() and (wsum > 0).all())
        C += (bs_scaled + np.log(np.maximum(wsum, 1e-300))).sum()
        wg = w_sg.sum(axis=1)
        healthy &= bool(np.isfinite(wg).all() and (wg > 0).all())
        lp = (bs_scaled + np.log(np.maximum(wg, 1e-300))).reshape(32, 4)
        mxg = lp.max(axis=1, keepdims=True)
        lse_g = mxg[:, 0] + np.log(np.exp(lp - mxg).sum(axis=1))
        C += lse_g.sum()
        C_g += lse_g.sum()
        v = v_t.sum(axis=1).reshape(32, 4).sum(axis=1)
        z = z_t.sum(axis=1).reshape(32, 4).sum(axis=1)
        healthy &= bool(np.isfinite(v).all() and np.isfinite(z).all()
                        and (z > 0).all())
        diag1 += (v / np.maximum(z, 1e-300)).sum() / ts
        healthy &= bool(np.isfinite(s_sl).all() and np.isfinite(s_sg).all()
                        and np.isfinite(p_o).all())

    cross = P @ S / ts - C * P.sum()
    diag = diag1 - C_g
    total = -cross + diag
    n_s = N_G + N_L
    n_loss_terms = N_T * n_s - min(N_T, n_s)
    loss = total / n_loss_terms
    healthy &= bool(np.isfinite(loss))
    return loss, healthy


def _numpy_loss(sg_full, sl_full, teacher, ts, tt):
    """Exact host fallback (never hit for sane input distributions)."""
    x = np.concatenate([sg_full, sl_full], axis=0).astype(np.float64) / ts
    lq = x - x.max(axis=1, keepdims=True)
    lq -= np.log(np.exp(lq).sum(axis=1, keepdims=True))
    y = teacher.astype(np.float64) / tt
    e = np.exp(y - y.max(axis=1, keepdims=True))
    p = e / e.sum(axis=1, keepdims=True)
    ce = -(p @ lq.T)
    n_t, n_s = ce.shape
    idx = np.arange(n_t)
    ce[idx, idx] = 0.0
    return ce.sum() / (n_t * n_s - min(n_t, n_s))


def kernel(out_student_global, out_student_local, out_teacher, center,
           temp_student, temp_teacher, cent_rate_m):
    out_student_global = np.asarray(out_student_global)
    out_student_local = np.asarray(out_student_local)
    out_teacher = np.asarray(out_teacher)
    center = np.asarray(center)
    ts = float(np.asarray(temp_student).reshape(-1)[0])
    tt = float(np.asarray(temp_teacher).reshape(-1)[0])

    teacher = out_teacher
    if np.any(center):
        teacher = out_teacher - center.reshape(1, -1).astype(np.float32)
    teacher_b = np.ascontiguousarray(teacher, dtype=ml_dtypes.bfloat16)
    sg_b = np.ascontiguousarray(out_student_global, dtype=ml_dtypes.bfloat16)
    sl_b = np.ascontiguousarray(out_student_local, dtype=ml_dtypes.bfloat16)

    # Safe exp bounds: strided-sample max + margin (bf16 rounding of the
    # true max is way below the margin).
    smax = max(float(sl_b.ravel()[::257].astype(np.float32).max()),
               float(sg_b.ravel()[::257].astype(np.float32).max()))
    b_s = smax + 1.0
    nbs = np.full((128, 1), -b_s / ts, np.float32)
    tmax = float(teacher_b.ravel()[::257].astype(np.float32).max())
    b_t = tmax + 1.0
    nbt = np.full((128, 1), -b_t / tt, np.float32)

    nc = _get_nc(ts, tt)
    in_maps = []
    for c in range(N_CORES):
        in_maps.append({
            "sl": sl_b[c * SL_ROWS:(c + 1) * SL_ROWS],
            "sg": sg_b[c * SG_ROWS:(c + 1) * SG_ROWS].reshape(128, DQ),
            "t": teacher_b[c * T_ROWS:(c + 1) * T_ROWS].reshape(128, DQ),
            "nbs": nbs,
            "nbt": nbt,
        })
    res = run_bass_kernel_spmd(nc, in_maps, core_ids=list(range(N_CORES)))
    loss, healthy = _merge(res.results, ts, tt, b_s / ts)
    if not healthy:
        loss = _numpy_loss(
            np.ascontiguousarray(out_student_global, dtype=np.float32),
            np.ascontiguousarray(out_student_local, dtype=np.float32),
            np.ascontiguousarray(teacher, dtype=np.float32), ts, tt)
    return np.float32(loss)
